# revision 6
# baseline (speedup 1.0000x reference)
"""Causal self-attention kernel for 8 Trainium2 NeuronCores.

y = CausalSelfAttention(x): B=2, T=2048, C=2048, 16 heads, head_dim=128,
fp32 in/out. Matmul operands in fp16 (same PE rate as fp32r at 1 col/
cycle, but half the DMA bytes / LDWEIGHTS time and full rate at small
free dims; rel_l2 ~7e-4 vs the 2e-2 gate).

Sharding (8 cores): core = (b, hg), b in {0,1} batch, hg in {0..3} head
group of 4 heads. Per core:
  phase 1: Q^T,K^T as [hd,T] (pass A) and V as [T,hd] (pass B) for its
           4 heads; weights prefetched across sync/scalar queues so the
           PE never waits on weight DMA.
  phase 2: causal attention per (head, 512-q-tile) in transposed layout:
           S^T = K Q^T (PE) -> exp (ACT, fused 1/sqrt(hd) scale) ->
           y^T += V^T E, den += 1^T E (PE, PSUM accum over k-tiles).
           Fine-grained causality: diagonal 128-wide k-tiles compute only
           columns q >= k-block (partial-width PSUM accumulation); only
           the 128x128 diagonal sub-block needs an elementwise triangular
           mask (DVE). Softmax normalization via 1/den = exp(-ln(den)) on
           the ACT engine (both funcs share one activation table). A 2-deep software pipeline (y/den
           matmuls trail the S matmuls by two k-tiles) hides the
           ACT-exp latency from the in-order PE queue.
  phase 3: partial c_proj per 128-token block, interleaved one q-tile
           behind phase 2 so the PE never drains; fp16 partial outputs.
Host: out[b] = sum of the 4 head-group fp16 partials (fp32 acc) + b_proj.

No collectives; one SPMD NEFF, per-core input data differs.
"""

import numpy as np
import concourse.bass as bass
import concourse.mybir as mybir
import concourse.tile as tile
from concourse.bass_utils import run_bass_kernel_spmd

B, T, C = 2, 2048, 2048
N_HEAD = 16
HD = 128
HPC = 4          # heads per core
HCOLS = HPC * HD  # 512
P = 128
QT = 512         # q-tile in attention
KT = 128         # k-tile in attention
NQT = T // QT    # 4
NCT = C // P     # 16 contraction tiles
SCALE = 1.0 / np.sqrt(HD)

F32 = mybir.dt.float32
F16 = mybir.dt.float16
MMDT = F16
NP_MMDT = np.float16


def build_nc(split_waits=True):
    nc = bass.Bass("TRN2", target_bir_lowering=False, debug=False)

    xT = nc.dram_tensor("xT", [C, T], MMDT, kind="ExternalInput").ap()
    wq = nc.dram_tensor("wq", [C, HCOLS], MMDT, kind="ExternalInput").ap()
    wk = nc.dram_tensor("wk", [C, HCOLS], MMDT, kind="ExternalInput").ap()
    wv = nc.dram_tensor("wv", [C, HCOLS], MMDT, kind="ExternalInput").ap()
    bq = nc.dram_tensor("bq", [P, HPC], F32, kind="ExternalInput").ap()
    bk = nc.dram_tensor("bk", [P, HPC], F32, kind="ExternalInput").ap()
    bv = nc.dram_tensor("bv", [P, HCOLS], F32, kind="ExternalInput").ap()
    wp = nc.dram_tensor("wp", [HCOLS, C], MMDT, kind="ExternalInput").ap()
    ones = nc.dram_tensor("ones", [P, P], MMDT, kind="ExternalInput").ap()
    tri = nc.dram_tensor("tri", [P, P], MMDT, kind="ExternalInput").ap()
    out = nc.dram_tensor("out", [T, C], F16, kind="ExternalOutput").ap()

    with tile.TileContext(nc) as tc:
        _build(tc, xT, wq, wk, wv, bq, bk, bv, wp, ones, tri, out)
    if split_waits:
        _split_matmul_waits(nc)
    return nc


def _split_matmul_waits(nc):
    """Lowered instructions fit only ONE sync-wait command (walrus: 'Too many
    sync wait commands'). Move excess waits onto preceding same-engine
    EventSemaphore instructions (which hold 2 waits each)."""
    n = 0
    for f in nc.m.functions:
        for b in f.blocks:
            patched = []
            changed = False
            for inst in b.instructions:
                si = inst.sync_info
                if (
                    not isinstance(inst, mybir.InstEventSemaphore)
                    and si is not None
                    and len(si.on_wait) > 1
                ):
                    waits = list(si.on_wait)
                    extra, keep = waits[:-1], waits[-1:]
                    for ci in range(0, len(extra), 2):
                        n += 1
                        patched.append(
                            mybir.InstEventSemaphore(
                                name=f"{inst.name}-wsplit{ci}",
                                engine=inst.engine,
                                ins=[],
                                outs=[],
                                sync_info=mybir.SyncInfo(
                                    on_wait=extra[ci:ci + 2], on_update=[]
                                ),
                            )
                        )
                    si.on_wait = keep
                    changed = True
                patched.append(inst)
            if changed:
                b.instructions = patched
    return n


def _build(tc, xT, wq, wk, wv, bq, bk, bv, wp, ones, tri, out):
    from contextlib import ExitStack

    nc = tc.nc
    Exp = mybir.ActivationFunctionType.Exp
    Ln = mybir.ActivationFunctionType.Ln
    Ident = mybir.ActivationFunctionType.Identity

    with ExitStack() as root:
        # ---- SBUF residents ----------------------------------------------
        res_qkv = root.enter_context(tc.tile_pool(name="res_qkv", bufs=1))
        qt_sb = res_qkv.tile([P, HPC, T], MMDT, tag="qt_sb")
        kt_sb = res_qkv.tile([P, HPC, T], MMDT, tag="kt_sb")
        v_sb = res_qkv.tile([P, NCT, HCOLS], MMDT, tag="v_sb")

        consts = root.enter_context(tc.tile_pool(name="consts", bufs=1))
        bq_sb = consts.tile([P, HPC], F32, tag="bq_sb")
        bk_sb = consts.tile([P, HPC], F32, tag="bk_sb")
        bv_sb = consts.tile([P, HCOLS], F32, tag="bv_sb")
        ones_sb = consts.tile([P, P], MMDT, tag="ones_sb")
        tri_sb = consts.tile([P, P], MMDT, tag="tri_sb")

        def load_consts():
            nc.gpsimd.dma_start(out=bq_sb[:, :], in_=bq)
            nc.gpsimd.dma_start(out=bk_sb[:, :], in_=bk)
            nc.gpsimd.dma_start(out=bv_sb[:, :], in_=bv)
            nc.gpsimd.dma_start(out=ones_sb[:, :], in_=ones)
            nc.gpsimd.dma_start(out=tri_sb[:, :], in_=tri)

        # weight slabs stay resident the whole kernel (bf16 makes it fit)
        wslab = root.enter_context(tc.tile_pool(name="wslab", bufs=1))
        wq_sb = wslab.tile([P, NCT, HCOLS], MMDT, tag="wq_sb")
        wk_sb = wslab.tile([P, NCT, HCOLS], MMDT, tag="wk_sb")
        wv_sb = wslab.tile([P, NCT, HCOLS], MMDT, tag="wv_sb")
        wp_sb = wslab.tile([P, HPC, C], MMDT, tag="wp_sb")

        wqr = wq.rearrange("(co ci) n -> ci co n", ci=P)
        wkr = wk.rearrange("(co ci) n -> ci co n", ci=P)
        wvr = wv.rearrange("(co ci) n -> ci co n", ci=P)
        wp_r = wp.rearrange("(ht p) c -> p ht c", p=P)

        # ---- phase 1: QKV projections ------------------------------------
        xpool = root.enter_context(tc.tile_pool(name="xt", bufs=8))
        with ExitStack() as ph1:
            pj_psum = ph1.enter_context(
                tc.tile_pool(name="pj_psum", bufs=8, space="PSUM")
            )

            PREF = 8

            def xt_dma(ci, tt, nm):
                t = xpool.tile([P, QT], MMDT, tag="xt", name=nm)
                eng = nc.sync if ci % 2 == 0 else nc.gpsimd
                eng.dma_start(
                    out=t[:, :],
                    in_=xT[ci * P:(ci + 1) * P, tt * QT:(tt + 1) * QT],
                )
                return t

            # first wq/wk chunks split across four queues to start fast
            W_CHUNKS = [(0, 1), (1, 2), (2, 4), (4, 8), (8, 16)]

            def w_chunk(i):
                sl = slice(*W_CHUNKS[i])
                eng = nc.sync if i < 2 else nc.gpsimd
                eng.dma_start(out=wq_sb[:, sl, :], in_=wqr[:, sl, :])
                nc.scalar.dma_start(out=wk_sb[:, sl, :], in_=wkr[:, sl, :])

            w_chunk(0)
            # first x tile on gpsimd so it transfers in parallel with the
            # first wq chunk on sync
            t0 = xpool.tile([P, QT], MMDT, tag="xt", name="xpre0")
            nc.gpsimd.dma_start(out=t0[:, :], in_=xT[0:P, 0:QT])
            pref = {(0, 0): t0, (0, 1): xt_dma(1, 0, "xpre1")}
            load_consts()
            w_chunk(1)
            pref.update({(0, ci): xt_dma(ci, 0, f"xpre{ci}")
                         for ci in range(2, PREF)})
            for i in range(2, 5):
                w_chunk(i)
            # wv prefetch rides the scalar queue behind wk during pass A
            for i in range(4):
                sl = slice(i * (NCT // 4), (i + 1) * (NCT // 4))
                nc.scalar.dma_start(out=wv_sb[:, sl, :], in_=wvr[:, sl, :])

            # pass A: Q and K (one xT stream, 8 psum banks)
            for tt in range(NQT):
                psq = [pj_psum.tile([P, QT], F32, tag="pj", name=f"pq{i}")
                       for i in range(HPC)]
                psk = [pj_psum.tile([P, QT], F32, tag="pj", name=f"pk{i}")
                       for i in range(HPC)]
                for ci in range(NCT):
                    xt_t = pref.pop((tt, ci), None)
                    if xt_t is None:
                        xt_t = xt_dma(ci, tt, f"xa{tt}_{ci}")
                    for h in range(HPC):
                        nc.tensor.matmul(
                            psq[h][:, :],
                            (wq_sb[:, ci, h * HD:(h + 1) * HD]),
                            (xt_t[:, :]),
                            start=(ci == 0),
                            stop=(ci == NCT - 1),
                        )
                    for h in range(HPC):
                        nc.tensor.matmul(
                            psk[h][:, :],
                            (wk_sb[:, ci, h * HD:(h + 1) * HD]),
                            (xt_t[:, :]),
                            start=(ci == 0),
                            stop=(ci == NCT - 1),
                        )
                for h in range(HPC):
                    nc.scalar.activation(
                        qt_sb[:, h, tt * QT:(tt + 1) * QT], psq[h][:, :],
                        Ident, bias=bq_sb[:, h:h + 1],
                    )
                for h in range(HPC):
                    nc.scalar.activation(
                        kt_sb[:, h, tt * QT:(tt + 1) * QT], psk[h][:, :],
                        Ident, bias=bk_sb[:, h:h + 1],
                    )

            # pass B: V (second xT stream)
            for ci in range(PREF):
                pref[("b", 0, ci)] = xt_dma(ci, 0, f"xbpre{ci}")
            # wp prefetch on the scalar queue during pass B
            for i in range(4):
                nc.scalar.dma_start(
                    out=wp_sb[:, i, :], in_=wp_r[:, i, :])
            for tt in range(NQT):
                psv = [pj_psum.tile([P, HCOLS], F32, tag="pj", name=f"pv{i}")
                       for i in range(4)]
                for ci in range(NCT):
                    xt_t = pref.pop(("b", tt, ci), None)
                    if xt_t is None:
                        xt_t = xt_dma(ci, tt, f"xb{tt}_{ci}")
                    for ts in range(4):
                        nc.tensor.matmul(
                            psv[ts][:, :],
                            (xt_t[:, ts * P:(ts + 1) * P]),
                            (wv_sb[:, ci, :]),
                            start=(ci == 0),
                            stop=(ci == NCT - 1),
                        )
                for ts in range(4):
                    nc.vector.tensor_add(
                        v_sb[:, tt * 4 + ts, :], psv[ts][:, :], bv_sb[:, :]
                    )

        # ---- phase 2 + 3: attention with interleaved c_proj --------------
        res_yt = root.enter_context(tc.tile_pool(name="res_yt", bufs=1))
        yt_sb = res_yt.tile([P, HPC, T], MMDT, tag="yt_sb")

        with ExitStack() as ph2:
            s_psum = ph2.enter_context(tc.tile_pool(name="s_psum", bufs=3, space="PSUM"))
            y_psum = ph2.enter_context(tc.tile_pool(name="y_psum", bufs=2, space="PSUM"))
            dc_psum = ph2.enter_context(tc.tile_pool(name="dc_psum", bufs=3, space="PSUM"))
            epool = ph2.enter_context(tc.tile_pool(name="epool", bufs=5))
            npool = ph2.enter_context(tc.tile_pool(name="npool", bufs=2))
            opool = ph2.enter_context(tc.tile_pool(name="opool", bufs=2))

            state = {}
            pending = []  # deque of (h, qt, kt, e_sb, qoff), depth 2

            def emit_yden(h, qt, kt, e_sb, qoff):
                nkt = 4 * qt + 4
                y_ps, den_ps = state[(h, qt)]
                # den before y: its stop releases the normalization chain
                # one matmul earlier
                nc.tensor.matmul(
                    den_ps[:, qoff:QT],
                    (ones_sb[:, :]),
                    (e_sb[:, qoff:QT]),
                    start=(kt == 0),
                    stop=(kt == nkt - 1),
                    skip_group_check=True,
                )
                nc.tensor.matmul(
                    y_ps[:, qoff:QT],
                    (v_sb[:, kt, h * HD:(h + 1) * HD]),
                    (e_sb[:, qoff:QT]),
                    start=(kt == 0),
                    stop=(kt == nkt - 1),
                    skip_group_check=True,
                )
                if kt == nkt - 1:
                    # 1/den = exp(-ln(den)) on ACT (~0.7us/op, same act
                    # table set as the softmax Exp) instead of the 3.3us
                    # DVE reciprocal that stalled short heads
                    lnd = npool.tile([P, QT], F32, tag="lnd",
                                     name=f"lnd{h}_{qt}")
                    nc.scalar.activation(lnd[:, :], den_ps[:, :], Ln)
                    rbc = npool.tile([P, QT], F32, tag="rbc",
                                     name=f"rbc{h}_{qt}")
                    nc.scalar.activation(rbc[:, :], lnd[:, :], Exp,
                                         scale=-1.0)
                    nc.vector.tensor_mul(
                        yt_sb[:, h, qt * QT:(qt + 1) * QT], y_ps[:, :],
                        rbc[:, :]
                    )
                    del state[(h, qt)]

            def emit_proj(qt):
                # c_proj for token blocks of q-tile qt (all 4 heads ready)
                for qi in range(4 * qt, 4 * qt + 4):
                    o_sb = opool.tile([P, C], F16, tag="o_sb",
                                      name=f"o{qi}")
                    for ct in range(C // QT):
                        cp = dc_psum.tile([P, QT], F32, tag="dc")
                        for h in range(HPC):
                            nc.tensor.matmul(
                                cp[:, :],
                                (yt_sb[:, h, qi * P:(qi + 1) * P]),
                                (wp_sb[:, h, ct * QT:(ct + 1) * QT]),
                                start=(h == 0),
                                stop=(h == HPC - 1),
                            )
                        nc.vector.tensor_copy(
                            o_sb[:, ct * QT:(ct + 1) * QT], cp[:, :]
                        )
                        if ct == 1:
                            # first half ships while the second computes
                            nc.sync.dma_start(
                                out=out[qi * P:(qi + 1) * P, 0:2 * QT],
                                in_=o_sb[:, 0:2 * QT])
                    nc.sync.dma_start(
                        out=out[qi * P:(qi + 1) * P, 2 * QT:C],
                        in_=o_sb[:, 2 * QT:C])

            for qt in range(NQT):
                for h in range(HPC):
                    nkt = 4 * qt + 4
                    state[(h, qt)] = (
                        y_psum.tile([P, QT], F32, tag="y", name=f"y{h}_{qt}"),
                        dc_psum.tile([P, QT], F32, tag="dc",
                                    name=f"den{h}_{qt}"),
                    )
                    for kt in range(nkt):
                        j = kt - 4 * qt
                        qoff = max(j, 0) * KT  # diagonal tiles: partial width
                        s_ps = s_psum.tile([P, QT], F32, tag="s")
                        nc.tensor.matmul(
                            s_ps[:, qoff:QT],
                            (kt_sb[:, h, kt * KT:(kt + 1) * KT]),
                            (qt_sb[:, h, qt * QT + qoff:(qt + 1) * QT]),
                            start=True,
                            stop=True,
                        )
                        if len(pending) >= 2:
                            emit_yden(*pending.pop(0))
                        e_sb = epool.tile([P, QT], MMDT, tag="e")
                        nc.scalar.activation(
                            e_sb[:, qoff:QT], s_ps[:, qoff:QT], Exp,
                            scale=float(SCALE)
                        )
                        if j >= 0:
                            # diagonal 128x128 sub-block: per-element
                            # causal mask (valid iff k_lane <= q_col)
                            nc.vector.tensor_mul(
                                e_sb[:, qoff:qoff + KT],
                                e_sb[:, qoff:qoff + KT], tri_sb[:, :]
                            )
                        pending.append((h, qt, kt, e_sb, qoff))
                    # after first head of this q-tile, emit previous
                    # q-tile's c_proj (its yt finished normalizing by now)
                    if h == 0 and qt > 0:
                        while pending:
                            emit_yden(*pending.pop(0))
                        emit_proj(qt - 1)
            while pending:
                emit_yden(*pending.pop(0))
            emit_proj(NQT - 1)


def make_core_inputs(x, W_attn, b_attn, W_proj, b_proj):
    """Host-side shard/prep. Returns list of 8 input dicts."""
    x = np.asarray(x, dtype=np.float32)
    W_attn = np.asarray(W_attn, dtype=np.float32)
    b_attn = np.asarray(b_attn, dtype=np.float32)
    W_proj = np.asarray(W_proj, dtype=np.float32)
    b_proj = np.asarray(b_proj, dtype=np.float32)

    in_maps = []
    for core in range(8):
        b, hg = divmod(core, 4)
        cs = slice(HCOLS * hg, HCOLS * hg + HCOLS)
        in_maps.append(
            {
                "xT": np.ascontiguousarray(x[b].T).astype(NP_MMDT),
                "wq": np.ascontiguousarray(
                    W_attn[:, 0 * C:1 * C][:, cs]).astype(NP_MMDT),
                "wk": np.ascontiguousarray(
                    W_attn[:, 1 * C:2 * C][:, cs]).astype(NP_MMDT),
                "wv": np.ascontiguousarray(
                    W_attn[:, 2 * C:3 * C][:, cs]).astype(NP_MMDT),
                "bq": np.ascontiguousarray(
                    b_attn[0 * C:1 * C][cs].reshape(HPC, HD).T
                ),
                "bk": np.ascontiguousarray(
                    b_attn[1 * C:2 * C][cs].reshape(HPC, HD).T
                ),
                "bv": np.ascontiguousarray(
                    np.broadcast_to(b_attn[2 * C:3 * C][cs], (P, HCOLS))
                ),
                "wp": np.ascontiguousarray(W_proj[cs, :]).astype(NP_MMDT),
                "ones": np.ones((P, P), dtype=NP_MMDT),
                "tri": np.triu(np.ones((P, P))).astype(NP_MMDT),
            }
        )
    return in_maps


_NC_CACHE = {}


def get_nc(split_waits=True):
    key = ("nc", split_waits)
    if key not in _NC_CACHE:
        _NC_CACHE[key] = build_nc(split_waits)
    return _NC_CACHE[key]


def kernel(x, W_attn, b_attn, W_proj, b_proj):
    in_maps = make_core_inputs(x, W_attn, b_attn, W_proj, b_proj)
    nc = get_nc()
    res = run_bass_kernel_spmd(nc, in_maps, core_ids=list(range(8)))
    parts = [np.asarray(r["out"], dtype=np.float32) for r in res.results]
    y = np.empty((B, T, C), dtype=np.float32)
    bpf = np.asarray(b_proj, dtype=np.float32)
    for b in range(B):
        y[b] = parts[4 * b] + parts[4 * b + 1] + parts[4 * b + 2] + parts[4 * b + 3]
        y[b] += bpf
    return y


if __name__ == "__main__":
    rng = np.random.default_rng(0)
    x = rng.standard_normal((B, T, C), dtype=np.float32)
    W_attn = rng.standard_normal((C, 3 * C), dtype=np.float32) / np.sqrt(C)
    b_attn = rng.standard_normal(3 * C).astype(np.float32) * 0.02
    W_proj = rng.standard_normal((C, C), dtype=np.float32) / np.sqrt(C)
    b_proj = rng.standard_normal(C).astype(np.float32) * 0.02
    y = kernel(x, W_attn, b_attn, W_proj, b_proj)
    print(y.shape, y.dtype, float(np.abs(y).mean()))
